# revision 18
# baseline (speedup 1.0000x reference)
"""Multi-head attention (B=4, N=2048, D=1024, H=16, DH=64) on 8 TRN2 NeuronCores.

Sharding: (batch x query-half) grid = 4x2 = 8 cores, zero collectives.
Each core computes q/k/v projections for its batch (k/v over the full
sequence, q over its 1024-query half), rotary, attention, and the output
projection for its disjoint [1024, 1024] slice of the output.

Engine plan (v4):
  - ScalarE runs the softmax exp stream back-to-back (the ~290us floor).
  - Sim (q.k) matmuls use 64-row PE tiling. Per kt-pair cycle the emission
    is [simE kt, simE kt+1, simO kt, simO kt+1]: consecutive T0/T8
    instructions overlap on the PE row tiles (HW-verified ~2x), and each
    head's pair feeds one 2-bank exp activation, so et tiles stay
    per-head bf16.
  - av chains run in-block, lagged behind the exp stream (E by one cycle,
    O by two); denominators ride as a ones-column in vaug (av M=65).
  - Reciprocal runs on an SBUF copy (custom-DVE ops misread PSUM on HW);
    broadcast via gpsimd partition_broadcast (no PE work).
  - All projection / rotary / transpose work is drip-fed as "filler"
    between cycles; the prefix (k0 + first q sts) borrows the sim psum
    pool so its projection chains stay double-buffered.

Per-core layouts:
  xt   [128, 8, 2048]  x[b].T, seq axis permuted so the core's q-half is first
  qTp  [128, 8, 1024]  packed col-major q: pair p = heads (2p, 2p+1) in
                       partition halves 0:64 / 64:128
  kT   [128, 8, 2048]  packed col-major k, same pair layout
  vaug [128, 16, 1040] row-major v with a ones column per head
  etE/etO [128, 16, 512] bf16 exp tiles per (head, qc)
"""
import sys

sys.path.insert(0, "/opt/trn_rl_repo")

import numpy as np
import ml_dtypes

import concourse.bass as bass
import concourse.bacc as bacc
import concourse.mybir as mybir
import concourse.tile as tile
from concourse.masks import make_identity
from contextlib import ExitStack

BF = mybir.dt.bfloat16
F32 = mybir.dt.float32
bf16 = ml_dtypes.bfloat16

P = 128
B, N, D = 4, 2048, 1024
H, DH = 16, 64
NQ = N // 2          # queries per core
DT = D // P          # 8 d-tiles
STK = N // P         # 16 seq tiles (k/v)
STQ = NQ // P        # 8 seq tiles (q)
F = 512              # matmul free dim
NPAIR = H // 2       # 8 head pairs
EXPF = mybir.ActivationFunctionType.Exp
SCALE = DH ** -0.5
VW = DH + 1          # v columns per head incl the ones column

_CACHED_NC = None


def build_nc():
    nc = bacc.Bacc("TRN2", debug=False)
    xt_d = nc.dram_tensor("xt", [D, N], BF, kind="ExternalInput")
    cos_d = nc.dram_tensor("cosr", [N, DH], BF, kind="ExternalInput")
    sin_d = nc.dram_tensor("sinr", [N, DH], BF, kind="ExternalInput")
    wqkv_d = nc.dram_tensor("wqkv", [D, 3 * D], BF, kind="ExternalInput")
    wout_d = nc.dram_tensor("wout", [D, D], BF, kind="ExternalInput")
    out_d = nc.dram_tensor("out", [NQ, D], F32, kind="ExternalOutput")

    with tile.TileContext(nc) as tc, ExitStack() as pc:
        pers = pc.enter_context(tc.tile_pool(name="pers", bufs=1))
        kT = pers.tile([P, NPAIR, N], BF, name="kT")
        qTp = pers.tile([P, NPAIR, NQ], BF, name="qTp")
        vaug = pers.tile([P, STK, H * VW], BF, name="vaug")
        aoT = pers.tile([P, DT, NQ], BF, name="aoT")
        cosr = pers.tile([P, STK, DH], BF, name="cosr")
        sinr = pers.tile([P, STK, DH], BF, name="sinr")
        xt = pers.tile([P, DT, N], BF, name="xt")
        ident = pers.tile([P, P], BF, name="ident")
        dummy = pers.tile([1, 2], F32, name="dummy")

        make_identity(nc, ident[:])
        nc.vector.memset(dummy[:], 0.0)
        for hh in range(H):
            nc.vector.memset(vaug[:, :, hh * VW + DH : hh * VW + DH + 1], 1.0)

        wp = pc.enter_context(tc.tile_pool(name="wp", bufs=2))
        rp = pc.enter_context(tc.tile_pool(name="rp", bufs=1))
        ep = pc.enter_context(tc.tile_pool(name="ep", bufs=3))
        npo = pc.enter_context(tc.tile_pool(name="npo", bufs=1))
        ob = pc.enter_context(tc.tile_pool(name="ob", bufs=1))
        simp = pc.enter_context(tc.tile_pool(name="simp", bufs=2, space="PSUM"))
        avp = pc.enter_context(tc.tile_pool(name="avp", bufs=2, space="PSUM"))
        pj = pc.enter_context(tc.tile_pool(name="pj", bufs=1, space="PSUM"))
        xqp = pc.enter_context(tc.tile_pool(name="xqp", bufs=1, space="PSUM"))

        # preload the exp table set so the first real exp doesn't eat ~2.7us
        nc.scalar.activation(dummy[:], dummy[:], EXPF)

        # -------------------- input DMAs --------------------------------
        wch_cache = {}

        def load_wch(tgt, ch):
            key = (tgt, ch)
            if key not in wch_cache:
                wch = wp.tile([P, DT, F], BF, tag="wch", name=f"w_{tgt}{ch}")
                src = wout_d if tgt == "o" else wqkv_d
                base = {"q": 0, "k": D, "v": 2 * D, "o": 0}[tgt]
                colbase = base + ch * F
                for a in range(DT):
                    nc.sync.dma_start(
                        wch[:, a],
                        src.ap()[a * P : (a + 1) * P, colbase : colbase + F],
                    )
                wch_cache[key] = wch
            return wch_cache[key]

        def load_xt_piece(piece):
            for a in range(DT):
                nc.sync.dma_start(
                    xt[:, a, piece * F : (piece + 1) * F],
                    xt_d.ap()[a * P : (a + 1) * P, piece * F : (piece + 1) * F],
                )

        # order: what the first projection blocks need comes first
        load_wch("k", 0)
        load_xt_piece(0)
        nc.sync.dma_start(cosr[:], cos_d.ap().rearrange("(t p) d -> p t d", p=P))
        nc.sync.dma_start(sinr[:], sin_d.ap().rearrange("(t p) d -> p t d", p=P))
        for piece in range(1, 4):
            load_xt_piece(piece)

        # -------------------- filler machinery --------------------------
        filler = []          # list of [name, generator]
        filler_done = set()

        def pull(n):
            k = 0
            while k < n and filler:
                try:
                    next(filler[0][1])
                    k += 1
                except StopIteration:
                    filler_done.add(filler[0][0])
                    filler.pop(0)

        def pull_until(name):
            while filler and name not in filler_done:
                pull(50)

        # -------------------- projection emitters -----------------------
        def proj_block(tgt, ch, st, prefix=False):
            wch = load_wch(tgt, ch)
            if prefix:
                # borrow a sim-pool slot (2 banks; use bank 0) so prefix
                # projection chains stay double-buffered
                ps = simp.tile([P, 2, F], F32, tag="sim", name="pps")[:, 0, :]
            else:
                ps = pj.tile([P, F], F32, tag="pj", name="pjt")
            for a in range(DT):
                nc.tensor.matmul(
                    ps, xt[:, a, st * P : (st + 1) * P], wch[:, a, :],
                    start=(a == 0), stop=(a == DT - 1),
                )
                yield
            psv = ps.rearrange("p (h d) -> p h d", d=DH)
            co = cosr[:, st : st + 1, :].broadcast_to([P, 8, DH])
            silo = sinr[:, st : st + 1, 0:32].broadcast_to([P, 8, 32])
            sihi = sinr[:, st : st + 1, 32:64].broadcast_to([P, 8, 32])
            if tgt == "v":
                vb = 8 * ch * VW
                rr = vaug[:, st, vb : vb + 8 * VW].rearrange(
                    "p (h d) -> p h d", d=VW
                )[:, :, 0:DH]
            else:
                rr = rp.tile([P, 8, DH], BF, tag="rr", bufs=1, name="rr")
            t2 = rp.tile([P, 8, DH], BF, tag="t2", bufs=1, name="t2")
            nc.vector.tensor_mul(rr[:, :, :], psv, co)
            nc.vector.tensor_mul(t2[:, :, 0:32], psv[:, :, 32:64], silo)
            nc.vector.tensor_mul(t2[:, :, 32:64], psv[:, :, 0:32], sihi)
            nc.vector.tensor_add(rr[:, :, :], rr[:, :, :], t2[:])
            yield
            if tgt != "v":
                xp = xqp.tile([P, 4, P], BF, tag="xq", name="xq")
                for j in range(4):
                    nc.tensor.transpose(
                        xp[:, j, :], rr[:, 2 * j : 2 * j + 2, :], ident[:]
                    )
                    yield
                dst = kT if tgt == "k" else qTp
                dslice = dst[:, 4 * ch : 4 * ch + 4, st * P : (st + 1) * P]
                if prefix:
                    nc.scalar.copy(dslice, xp[:])  # scalar idle before exp
                else:
                    nc.vector.tensor_copy(dslice, xp[:])
                yield

        progress = {}

        def proj_chunk(tgt, ch, sts, prefix=False):
            for st in sts:
                yield from proj_block(tgt, ch, st, prefix=prefix)
                progress[(tgt, ch)] = st + 1

        def outproj_qc(qc):
            for ch in range(2):
                wch = load_wch("o", ch)
                yield
                for qt in range(qc * 4, qc * 4 + 4):
                    ps = pj.tile([P, F], F32, tag="pj", name="opt")
                    for a in range(DT):
                        nc.tensor.matmul(
                            ps, aoT[:, a, qt * P : (qt + 1) * P], wch[:, a, :],
                            start=(a == 0), stop=(a == DT - 1),
                        )
                        yield
                    o = ob.tile([P, F], F32, tag="o", name="ot")
                    nc.vector.tensor_copy(o[:], ps)
                    nc.sync.dma_start(
                        out_d.ap()[qt * P : (qt + 1) * P, ch * F : (ch + 1) * F],
                        o[:],
                    )
                    yield

        filler.extend([
            ["k0a", proj_chunk("k", 0, range(2), prefix=True)],
            ["q0a", proj_chunk("q", 0, range(4), prefix=True)],
            ["k0b", proj_chunk("k", 0, range(2, STK), prefix=True)],
            ["v0", proj_chunk("v", 0, range(STK))],
            ["q0b", proj_chunk("q", 0, range(4, STQ))],
            ["k1", proj_chunk("k", 1, range(STK))],
            ["q1", proj_chunk("q", 1, range(STQ))],
            ["v1", proj_chunk("v", 1, range(STK))],
        ])

        # -------------------- attention pipeline ------------------------
        av_queue = []

        def make_av_pair(p, qc, etE, etO):
            """av + norm for both heads of pair p. Two chains (E in bank A,
            O in bank B) advance as their exp acts are emitted; norm per
            head at chain end."""
            vchunk = "v0" if p < 4 else "v1"
            st_ = {"ktE": 0, "ktO": 0, "apE": None, "apO": None,
                   "acts": 0, "done": False}

            def norm(h, ap_):
                # reciprocal_approx_fast only works at partition base 0 on
                # HW (probe-verified); route the denom row through p0
                rc = npo.tile([1, F], F32, tag="rc", name="rc")
                rcr = npo.tile([1, F], F32, tag="rcr", name="rcr")
                nb = npo.tile([DH, F], F32, tag="nb", name="nb")
                nc.vector.tensor_copy(rc[0:1, :], ap_[DH : DH + 1, :])
                nc.vector.reciprocal_approx_fast(rcr[0:1, :], rc[0:1, :])
                nc.gpsimd.partition_broadcast(nb[:], rcr[0:1, :])
                hp = DH * (h % 2)
                nc.vector.tensor_mul(
                    aoT[hp : hp + DH, h // 2, qc * F : (qc + 1) * F],
                    ap_[0:DH, :],
                    nb[:],
                )

            def chain(which, et, h, budget, cap):
                kt = st_["kt" + which]
                ap_ = st_["ap" + which]
                n = 0
                while n < budget and kt < min(cap, STK):
                    if kt == 0:
                        ap_ = avp.tile([DH + 1, F], F32, tag="av", name="avt")
                        st_["ap" + which] = ap_
                    nc.tensor.matmul(
                        ap_,
                        vaug[:, kt, h * VW : h * VW + DH + 1],
                        et[:, kt, :],
                        start=(kt == 0), stop=(kt == STK - 1),
                    )
                    kt += 1
                    n += 1
                    if kt == STK:
                        norm(h, ap_)
                st_["kt" + which] = kt
                return n

            def run(budget):
                if st_["done"]:
                    return
                if vchunk not in filler_done:
                    return
                capE = st_["acts"]          # kts with emitted E acts
                capO = max(0, st_["acts"] - 2)  # O lags one cycle more
                b = chain("E", etE, 2 * p, (budget + 1) // 2, capE)
                chain("O", etO, 2 * p + 1, budget - b, capO)
                if st_["ktE"] >= STK and st_["ktO"] >= STK:
                    st_["done"] = True

            def force():
                pull_until(vchunk)
                st_["acts"] = 2 * STK
                run(4 * STK)

            run.state = st_
            run.force = force
            return run

        def do_sim_block(p, qc):
            # ep bufs=3 rotation is safe only if older av pairs are done
            while len(av_queue) > 1:
                av_queue[0].force()
                av_queue.pop(0)
            etE = ep.tile([P, STK, F], BF, tag="et", name="etE")
            etO = ep.tile([P, STK, F], BF, tag="et", name="etO")
            av = make_av_pair(p, qc, etE, etO)
            av_queue.append(av)
            kch = 0 if p < 4 else 1
            for kt2 in range(STK // 2):
                kt = 2 * kt2
                # av + filler first: gives exp acts time to drain the psum
                # tiles this cycle's sims will reuse
                before = sum(
                    a.state["ktE"] + a.state["ktO"] for a in av_queue
                )
                for a in av_queue:
                    a(4)
                emitted = sum(
                    a.state["ktE"] + a.state["ktO"] for a in av_queue
                ) - before
                if av_queue and av_queue[0].state["done"]:
                    av_queue.pop(0)
                pull(13 - min(4, emitted))
                while progress.get(("k", kch), 0) < kt + 2 and filler:
                    pull(20)
                spE = simp.tile([P, 2, F], F32, tag="sim", name="simE")
                spO = simp.tile([P, 2, F], F32, tag="sim", name="simO")
                for i in range(2):
                    # T0 then T8 back-to-back: the pair executes concurrently
                    nc.tensor.matmul(
                        spE[:, i, :],
                        kT[0:DH, p, (kt + i) * P : (kt + i + 1) * P],
                        qTp[0:DH, p, qc * F : (qc + 1) * F],
                        start=True, stop=True,
                    )
                    nc.tensor.matmul(
                        spO[:, i, :],
                        kT[DH:P, p, (kt + i) * P : (kt + i + 1) * P],
                        qTp[DH:P, p, qc * F : (qc + 1) * F],
                        start=True, stop=True,
                    )
                nc.scalar.activation(
                    etE[:, kt : kt + 2, :], spE[:], EXPF, scale=SCALE
                )
                nc.scalar.activation(
                    etO[:, kt : kt + 2, :], spO[:], EXPF, scale=SCALE
                )
                av.state["acts"] += 2

        # block schedule: group 0 = pairs 0-3 (heads 0-7), group 1 = pairs
        # 4-7; qc-major within each group so required q sub-chunks arrive
        # with slack and qc0's out-projection hides under the last blocks.
        blocks = [(p, qc) for qc in range(2) for p in range(4)]
        blocks += [(p, qc) for qc in range(2) for p in range(4, 8)]

        pull_until("k0a")
        pull_until("q0a")

        for bi, (p, qc) in enumerate(blocks):
            need = (["q0a"] if qc == 0 else ["q0b"]) if p < 4 else ["q1"]
            for name in need:
                pull_until(name)
            if bi == 13:
                filler.insert(0, ["op0", outproj_qc(0)])
            do_sim_block(p, qc)

        while av_queue:
            av_queue[0].force()
            av_queue.pop(0)
        filler.append(["op1", outproj_qc(1)])
        while filler:
            pull(100)
    nc.compile()
    return nc


def prep_inputs(x, rotary_pos_emb):
    """Per-core input maps. Core c = b*2 + qh."""
    freqs = np.asarray(rotary_pos_emb, dtype=np.float32)
    cos = np.cos(freqs)
    sin = np.sin(freqs)
    sin_folded = sin.copy()
    sin_folded[:, 0:32] = -sin_folded[:, 0:32]
    x = np.asarray(x, dtype=np.float32)
    in_maps = []
    for c in range(8):
        b, qh = c // 2, c % 2
        perm = np.roll(np.arange(N), -qh * NQ)
        in_maps.append(
            {
                "xt": np.ascontiguousarray(x[b].T[:, perm]).astype(bf16),
                "cosr": np.ascontiguousarray(cos[perm]).astype(bf16),
                "sinr": np.ascontiguousarray(sin_folded[perm]).astype(bf16),
            }
        )
    return in_maps


def kernel(x, mask, rotary_pos_emb, W_qkv, W_out):
    global _CACHED_NC
    from concourse.bass_utils import run_bass_kernel_spmd

    if _CACHED_NC is None:
        _CACHED_NC = build_nc()
    nc = _CACHED_NC

    wqkv_b = np.asarray(W_qkv, dtype=np.float32).astype(bf16)
    wout_b = np.asarray(W_out, dtype=np.float32).astype(bf16)
    in_maps = prep_inputs(x, rotary_pos_emb)
    for m in in_maps:
        m["wqkv"] = wqkv_b
        m["wout"] = wout_b

    res = run_bass_kernel_spmd(nc, in_maps, core_ids=list(range(8)))
    out = np.empty((B, N, D), dtype=np.float32)
    for c in range(8):
        b, qh = c // 2, c % 2
        out[b, qh * NQ : (qh + 1) * NQ, :] = res.results[c]["out"]
    return out
